# revision 1
# baseline (speedup 1.0000x reference)
"""DaGCN on 8 Trainium2 NeuronCores (Bass SPMD).

Strategy (graph/data parallel, nodes sharded 8 ways):
  * Each core owns a 6250-node shard (padded to 6272 = 49*128).
  * Feature transforms s = x @ W run as bf16 PE matmuls on host-transposed
    x shards; the resulting per-shard tables are AllGather'ed so every core
    holds the full [50176, 128] bf16 node-feature tables in its DRAM.
  * Edges are assigned to the core owning dst. Per (adjacency, src-half)
    they are sorted by dst block (128 nodes), each block's run padded to
    whole 128-edge chunks. dma_gather (1024 idxs/call) fetches s[src] as
    bf16 messages, edge-on-partition.
  * segment_sum runs on the TensorEngine: per 128-edge chunk a one-hot
    lhsT [128 edges x 128 dst-cols] holding ew (built on DVE from an iota
    compare) is matmul'ed with the message chunk, accumulating each dst
    block in PSUM. No scatter-add (HW races on duplicate indices).
  * Gating/normalization math runs on DVE/ACT over [128, 49, F] shard
    layouts entirely in SBUF.
"""

import math
from contextlib import ExitStack

import ml_dtypes
import numpy as np

import concourse.bacc as bacc
import concourse.bass as bass
import concourse.mybir as mybir
from concourse.bass_utils import run_bass_kernel_spmd

F32 = mybir.dt.float32
BF16 = mybir.dt.bfloat16
I16 = mybir.dt.int16
AOP = mybir.AluOpType
ACT = mybir.ActivationFunctionType

NCORES = 8
N = 50000
NFEAT, NHID, NCLASS = 256, 64, 32
S_CALL = 1024          # idxs per dma_gather call (HW-validated; 2048 hangs)
CALL_CHUNKS = S_CALL // 128
RING = 8               # gather/onehot ring depth (in calls)
NPSUM = 4              # psum block-accumulator ring


def _wrap16(a):
    """[n] int16 -> [128, n//16]: idx i at [i%16, i//16], replicated x8."""
    n = a.shape[0]
    w = a.reshape(n // 16, 16).T.astype(np.int16)
    return np.tile(w, (8, 1)).copy()


def _chunkwrap(a, dtype):
    """[n] -> [128, n//128]: edge i at [i%128, i//128]."""
    n = a.shape[0]
    return np.ascontiguousarray(a.reshape(n // 128, 128).T.astype(dtype))


def _prep_adjacency(src, dst, ew, S, SP, NB, HSPLIT, NROWS):
    """Bucket edges by dst core/block/src-half; returns per-core arrays + CPBs."""
    src = np.asarray(src).astype(np.int64)
    dst = np.asarray(dst).astype(np.int64)
    ew = np.asarray(ew).astype(np.float32)
    core = dst // S
    row = (src // S) * SP + (src % S)       # padded table row
    half = (row >= HSPLIT).astype(np.int64)
    dstrel = dst - core * S
    blk = dstrel // 128
    col = dstrel % 128

    percore = []
    counts = np.zeros((NCORES, 2, NB), np.int64)
    for k in range(NCORES):
        m = core == k
        e = np.lexsort((blk[m], half[m]))   # sort by (half, block)
        r, h, b, c, w = row[m][e], half[m][e], blk[m][e], col[m][e], ew[m][e]
        percore.append((r, h, b, c, w))
        for hh in range(2):
            mm = h == hh
            counts[k, hh] = np.bincount(b[mm], minlength=NB)

    cpb_lo = int(np.ceil(counts[:, 0].max() / 128))
    cpb_hi = int(np.ceil(counts[:, 1].max() / 128))
    cpb_lo = max(cpb_lo, 1)
    cpb_hi = max(cpb_hi, 1)
    ch_lo = -(-NB * cpb_lo // CALL_CHUNKS) * CALL_CHUNKS
    ch_hi = -(-NB * cpb_hi // CALL_CHUNKS) * CALL_CHUNKS
    nslot = (ch_lo + ch_hi) * 128

    out = []
    for k in range(NCORES):
        r, h, b, c, w = percore[k]
        gidx = np.zeros(nslot, np.int64)
        dcol = np.zeros(nslot, np.int64)
        eww = np.zeros(nslot, np.float32)
        for hh, cpb, base_ch, rowbase in ((0, cpb_lo, 0, 0), (1, cpb_hi, ch_lo, HSPLIT)):
            mm = h == hh
            rr, bb, cc, ww = r[mm], b[mm], c[mm], w[mm]
            # position within block run (edges already sorted by block)
            cnt = counts[k, hh]
            offs = np.concatenate(([0], np.cumsum(cnt)))[:-1]
            pos = np.arange(rr.shape[0]) - offs[bb]
            slot = (base_ch + bb * cpb) * 128 + pos
            gidx[slot] = rr - rowbase
            dcol[slot] = cc
            eww[slot] = ww
        out.append((
            _wrap16(gidx),
            _chunkwrap(dcol, np.float32),
            _chunkwrap(eww, np.float32),
        ))
    return out, cpb_lo, cpb_hi, ch_lo, ch_hi, nslot


class Ctr:
    def __init__(self, sem, step=1):
        self.sem, self.n, self.step = sem, 0, step

    def inc(self, inst):
        inst.then_inc(self.sem, self.step)
        self.n += self.step
        return self.n


def _build(S, SP, NB, NROWS, HSPLIT, adjmeta, scalars):
    """adjmeta: {a: (ch_lo, ch_hi, nslot)}; scalars: g1b,g2b,h1b,h2b floats."""
    nc = bacc.Bacc("TRN2", num_devices=NCORES, num_swdge_queues=2)
    g1b, g2b, h1b, h2b = scalars
    ncal_max = max((m[0] + m[1]) // CALL_CHUNKS for m in adjmeta.values())
    nslot_max = max(m[2] for m in adjmeta.values())
    nch_max = nslot_max // 128

    # ---------------- I/O ----------------
    din = {}
    for v in ("xt1a", "xt1b", "xt2a", "xt2b"):
        din[v] = nc.dram_tensor(v, [128, 2, SP], BF16, kind="ExternalInput")
    din["w1a"] = nc.dram_tensor("w1a", [128, 2, NHID], BF16, kind="ExternalInput")
    din["w1b"] = nc.dram_tensor("w1b", [128, 2, NHID], BF16, kind="ExternalInput")
    din["w2"] = nc.dram_tensor("w2", [128, 64], BF16, kind="ExternalInput")
    din["iota"] = nc.dram_tensor("iota", [128, 128], BF16, kind="ExternalInput")
    din["idf"] = nc.dram_tensor("idf", [128, 128], F32, kind="ExternalInput")
    din["idb"] = nc.dram_tensor("idb", [128, 128], BF16, kind="ExternalInput")
    din["g1w"] = nc.dram_tensor("g1w", [128, 128], F32, kind="ExternalInput")
    din["g2w"] = nc.dram_tensor("g2w", [128, 128], F32, kind="ExternalInput")
    din["h1w"] = nc.dram_tensor("h1w", [128, 64], F32, kind="ExternalInput")
    din["h2w"] = nc.dram_tensor("h2w", [128, 64], F32, kind="ExternalInput")
    din["b1r"] = nc.dram_tensor("b1r", [128, 128], F32, kind="ExternalInput")
    din["b2r"] = nc.dram_tensor("b2r", [128, 64], F32, kind="ExternalInput")
    for a in (1, 2):
        ns = adjmeta[a][2]
        din[f"gidx{a}"] = nc.dram_tensor(f"gidx{a}", [128, ns // 16], I16, kind="ExternalInput")
        din[f"dst{a}"] = nc.dram_tensor(f"dst{a}", [128, ns // 128], F32, kind="ExternalInput")
        din[f"eww{a}"] = nc.dram_tensor(f"eww{a}", [128, ns // 128], F32, kind="ExternalInput")
    out_o = nc.dram_tensor("out_o", [SP, NCLASS], F32, kind="ExternalOutput")
    p1_o = nc.dram_tensor("p1_o", [SP, NCLASS], F32, kind="ExternalOutput")
    p2_o = nc.dram_tensor("p2_o", [SP, NCLASS], F32, kind="ExternalOutput")

    t_in = {t: nc.dram_tensor(f"t{t}in", [SP, 128], BF16) for t in (1, 2, 3)}
    t_full = {t: nc.dram_tensor(f"t{t}full", [NROWS, 128], BF16, addr_space="Shared")
              for t in (1, 2, 3)}

    ctx = ExitStack()
    sb = lambda name, shape, dt: ctx.enter_context(nc.sbuf_tensor(name, shape, dt))
    ps = lambda name, shape: ctx.enter_context(nc.psum_tensor(name, shape, F32))
    sem = lambda name: ctx.enter_context(nc.semaphore(name))

    # ---------------- SBUF ----------------
    c_w1a = sb("c_w1a", [128, 2, NHID], BF16)
    c_w1b = sb("c_w1b", [128, 2, NHID], BF16)
    c_w2 = sb("c_w2", [128, 64], BF16)
    c_iota = sb("c_iota", [128, 128], BF16)
    c_idf = sb("c_idf", [128, 128], F32)
    c_idb = sb("c_idb", [128, 128], BF16)
    c_g1w = sb("c_g1w", [128, 128], F32)
    c_g2w = sb("c_g2w", [128, 128], F32)
    c_h1w = sb("c_h1w", [128, 64], F32)
    c_h2w = sb("c_h2w", [128, 64], F32)
    c_b1r = sb("c_b1r", [128, 128], F32)
    c_b2r = sb("c_b2r", [128, 64], F32)

    sT = sb("sT", [128, SP], BF16)
    tstage = sb("tstage", [128, NB, 128], BF16)
    agg1 = sb("agg1", [128, NB, 128], F32)
    agg2 = sb("agg2", [128, NB, 128], F32)
    tmp = sb("tmp", [128, NB, 128], F32)
    xtt = sb("xtt", [128, 128], BF16)
    lamv = {nm: sb(nm, [128, NB], F32)
            for nm in ("l1", "l2", "lsum", "w0", "w1")}
    cbias = sb("cbias", [128, 4], F32)
    sbA = ExitStack()
    xta = sbA.enter_context(nc.sbuf_tensor("xta", [128, 2, SP], BF16))
    xtb = sbA.enter_context(nc.sbuf_tensor("xtb", [128, 2, SP], BF16))

    psA = ExitStack()
    mm_ps = [psA.enter_context(nc.psum_tensor(f"mm_ps{i}", [128, 512], F32))
             for i in range(2)]
    trb_ps = [psA.enter_context(nc.psum_tensor(f"trb_ps{i}", [128, 128], BF16))
              for i in range(2)]

    io = Ctr(sem("io"), 16)        # sync-engine DMAs
    gsems = [Ctr(sem(f"g{i}"), 16) for i in range(RING)]  # per-ring-slot gathers
    ccs = [Ctr(sem(f"cc{i}"), 1) for i in range(3)]   # one sem per collective
    pe = Ctr(sem("pe"), 1)         # PE milestones
    dv = Ctr(sem("dv"), 1)         # DVE milestones
    ac = Ctr(sem("ac"), 1)         # ACT milestones

    SY, PE, DV, AC, GP = nc.sync, nc.tensor, nc.vector, nc.scalar, nc.gpsimd

    def fence():
        # sync engine waits for all its issued DMAs: later cross-engine
        # io-threshold waits become unambiguous (no completion reordering).
        SY.wait_ge(io.sem, io.n)

    # =========== Phase A: constants + s tables ===========
    for bi, bval in enumerate((g1b, g2b, h1b, h2b)):
        nc.vector.memset(cbias[:, bi:bi + 1], float(bval))
    for name, t in (("w1a", c_w1a), ("w1b", c_w1b), ("w2", c_w2), ("iota", c_iota),
                    ("idf", c_idf), ("idb", c_idb), ("g1w", c_g1w), ("g2w", c_g2w),
                    ("h1w", c_h1w), ("h2w", c_h2w), ("b1r", c_b1r), ("b2r", c_b2r)):
        io.inc(SY.dma_start(t[:], din[name][:]))
    consts_io = io.n

    nsl = [(j * 512, min(512, SP - j * 512)) for j in range((SP + 511) // 512)]

    def s_table(tbl, va, vb, wa, wb, pe_wait_extra):
        """matmul s = [x_va@W1a | x_vb@W1b] -> transpose -> tstage -> DMA t_in."""
        io.inc(SY.dma_start(xta[:], din[va][:]))
        io.inc(SY.dma_start(xtb[:], din[vb][:]))
        xload = io.n
        copies = []
        for j, (o, n) in enumerate(nsl):
            p = mm_ps[j % 2]
            if j == 0:
                PE.wait_ge(io.sem, xload)
                if pe_wait_extra is not None:
                    PE.wait_ge(dv.sem, pe_wait_extra)
            if j >= 2 and copies[j - 2] is not None:
                PE.wait_ge(dv.sem, copies[j - 2])
            for xt, w, prow in ((xta, wa, 0), (xtb, wb, 64)):
                for cch in range(2):
                    last = PE.matmul(p[prow:prow + 64, 0:n], w[:, cch, :],
                                     xt[:, cch, o:o + n],
                                     start=(cch == 0), stop=(cch == 1))
            pe.inc(last)
            pev = pe.n
            DV.wait_ge(pe.sem, pev)
            cp = DV.tensor_copy(sT[:, o:o + n], p[:, 0:n])
            dv.inc(cp)
            copies.append(dv.n)
        # transposes into tstage
        trc = {}
        for t in range(NB):
            p = trb_ps[t % 2]
            PE.wait_ge(dv.sem, copies[-1])
            if t >= 2:
                PE.wait_ge(dv.sem, trc[t - 2])
            pe.inc(PE.transpose(p[:], sT[:, t * 128:(t + 1) * 128], c_idb[:]))
            DV.wait_ge(pe.sem, pe.n)
            dv.inc(DV.tensor_copy(tstage[:, t, :], p[:]))
            trc[t] = dv.n
        SY.wait_ge(dv.sem, dv.n)
        io.inc(SY.dma_start(
            t_in[tbl][:].rearrange("(t p) f -> p t f", p=128), tstage[:]))
        fence()
        return io.n, pe.n

    t1_io, t1_pe = s_table(1, "xt1a", "xt1b", c_w1a, c_w1b, None)
    # table2 reuses xta/xtb: its x DMAs must wait for table1's matmuls;
    # emit the waits on the sync engine before the loads.
    SY.wait_ge(pe.sem, t1_pe)
    # tstage reuse: table2's transpose copies (DVE) wait t1in DMA done
    DV.wait_ge(io.sem, t1_io)
    t2_io, t2_pe = s_table(2, "xt2a", "xt2b", c_w1a, c_w1b, None)

    pe_phaseA = pe.n
    GP.wait_ge(io.sem, t1_io)
    ccs[0].inc(GP.collective_compute(
        "AllGather", AOP.bypass, replica_groups=[list(range(NCORES))],
        ins=[t_in[1][:]], outs=[t_full[1][:]]))
    GP.wait_ge(io.sem, t2_io)
    ccs[1].inc(GP.collective_compute(
        "AllGather", AOP.bypass, replica_groups=[list(range(NCORES))],
        ins=[t_in[2][:]], outs=[t_full[2][:]]))

    # =========== edge pass machinery ===========
    psA.close()  # phase-A PSUM freed; per-engine program order makes reuse safe
    sbA.close()  # xta/xtb freed -> reused by edge buffers (guarded by waits below)
    blk_ps = [ps(f"blk_ps{i}", [128, 128]) for i in range(NPSUM)]
    tr_ps = [ps(f"tr_ps{i}", [128, 128]) for i in range(2)]
    prop1 = sb("prop1", [128, NB, 64], F32)
    prop2 = sb("prop2", [128, NB, 64], F32)
    msg = sb("msg", [128, RING * CALL_CHUNKS, 128], BF16)
    ohr = sb("ohr", [128, RING * CALL_CHUNKS, 128], BF16)
    gidx_sb = sb("gidx_sb", [128, nslot_max // 16], I16)
    dst_sb = sb("dst_sb", [128, nch_max], F32)
    ew_sb = sb("ew_sb", [128, nch_max], F32)
    gcall = [0]      # global gather call counter
    pe_cons_vals = []
    npass = [0]
    psum_last = [None] * NPSUM  # (sem, val) of last copy freeing each psum slot

    def edge_pass(adj, table, F_rhs, dest, cc_need, ch_lo, ch_hi, cpb_lo, cpb_hi,
                  add_mode):
        """One (layer, adjacency) pass: lo half then hi half."""
        ns = (ch_lo + ch_hi) * 128
        # WAR: don't overwrite idx arrays while a previous pass still reads
        # them, nor the freed xta/xtb space while phase-A PE still reads it
        for gs in gsems:
            SY.wait_ge(gs.sem, gs.n)
        SY.wait_ge(dv.sem, dv.n)
        if npass[0] == 0:
            SY.wait_ge(pe.sem, pe_phaseA)
        io.inc(SY.dma_start(gidx_sb[:, 0:ns // 16], din[f"gidx{adj}"][:]))
        io.inc(SY.dma_start(dst_sb[:, 0:ns // 128], din[f"dst{adj}"][:]))
        io.inc(SY.dma_start(ew_sb[:, 0:ns // 128], din[f"eww{adj}"][:]))
        fence()
        idx_io = io.n

        lo_copy_ac = {}
        GP.wait_ge(ccs[cc_need].sem, 1)
        if npass[0] == 0:
            GP.wait_ge(pe.sem, pe_phaseA)   # msg ring aliases freed xta/xtb
            DV.wait_ge(pe.sem, pe_phaseA)   # ohr ring likewise
        npass[0] += 1
        for half, ch, cpb, base in ((0, ch_lo, cpb_lo, 0), (1, ch_hi, cpb_hi, HSPLIT)):
            ch0 = 0 if half == 0 else ch_lo  # chunk offset in the arrays
            tab = table[base:NROWS] if half == 1 else table[0:HSPLIT]
            blk_of = lambda c: min(c // cpb, NB - 1)
            endc = lambda b: (b + 1) * cpb - 1 if b < NB - 1 else ch - 1
            for j in range(ch // CALL_CHUNKS):
                rj = (gcall[0] % RING) * CALL_CHUNKS
                GP.wait_ge(io.sem, idx_io)
                if len(pe_cons_vals) >= RING:
                    GP.wait_ge(pe.sem, pe_cons_vals[-RING])
                gslot = gcall[0] % RING
                g = GP.dma_gather(
                    msg[:, rj:rj + CALL_CHUNKS, :], tab,
                    gidx_sb[:, (ch0 * 8 + j * S_CALL // 16):(ch0 * 8 + (j + 1) * S_CALL // 16)],
                    S_CALL, S_CALL, 128, queue_num=gcall[0] % 2)
                gsems[gslot].inc(g)
                gv = gsems[gslot].n
                # onehot build
                DV.wait_ge(io.sem, idx_io)
                if len(pe_cons_vals) >= RING:
                    DV.wait_ge(pe.sem, pe_cons_vals[-RING])
                cbase = ch0 + j * CALL_CHUNKS
                for c8 in range(CALL_CHUNKS):
                    ts = DV.tensor_scalar(
                        ohr[:, rj + c8, :], c_iota[:],
                        dst_sb[:, cbase + c8:cbase + c8 + 1],
                        ew_sb[:, cbase + c8:cbase + c8 + 1],
                        op0=AOP.is_equal, op1=AOP.mult)
                dv.inc(ts)
                ohv = dv.n
                # matmuls
                PE.wait_ge(gsems[gslot].sem, gv)
                PE.wait_ge(dv.sem, ohv)
                last_was_end = False
                for c8 in range(CALL_CHUNKS):
                    c = j * CALL_CHUNKS + c8
                    b = blk_of(c)
                    slot = b % NPSUM
                    p = blk_ps[slot]
                    st = (c == b * cpb)
                    if st and psum_last[slot] is not None:
                        eng, val = psum_last[slot]
                        PE.wait_ge(dv.sem if eng == "dv" else ac.sem, val)
                    mmi = PE.matmul(p[:, 0:F_rhs],
                                    ohr[:, rj + c8, :],
                                    msg[:, rj + c8, 0:F_rhs],
                                    start=st, stop=(c == endc(b)))
                    last_was_end = (c == endc(b))
                    if last_was_end:
                        pe.inc(mmi)
                        if add_mode or half == 1:
                            DV.wait_ge(pe.sem, pe.n)
                            DV.wait_ge(ac.sem, lo_copy_ac[b])
                            cpi = DV.tensor_tensor(dest[:, b, 0:F_rhs],
                                                   dest[:, b, 0:F_rhs],
                                                   p[:, 0:F_rhs], op=AOP.add)
                            dv.inc(cpi)
                            psum_last[slot] = ("dv", dv.n)
                        else:
                            AC.wait_ge(pe.sem, pe.n)
                            cpi = AC.activation(dest[:, b, 0:F_rhs],
                                                p[:, 0:F_rhs], ACT.Copy)
                            ac.inc(cpi)
                            psum_last[slot] = ("ac", ac.n)
                            lo_copy_ac[b] = ac.n
                if not last_was_end:
                    pe.inc(mmi)
                pe_cons_vals.append(pe.n)
                gcall[0] += 1
        return dv.n

    m1 = adjmeta[1]
    m2 = adjmeta[2]
    edge_pass(1, t_full[1], 128, agg1, 0, m1[0], m1[1], m1[3], m1[4], False)
    edge_pass(2, t_full[2], 128, agg2, 1, m2[0], m2[1], m2[3], m2[4], False)

    # =========== Phase C: mid gating + L2 table ===========
    DV.drain()
    b1b = c_b1r[:, None, :].broadcast_to([128, NB, 128])
    DV.tensor_tensor(agg1[:], agg1[:], b1b, op=AOP.add)
    DV.tensor_tensor(agg2[:], agg2[:], b1b, op=AOP.add)
    DV.drain()
    DV.tensor_scalar(agg1[:], agg1[:], 0.0, None, op0=AOP.max)
    DV.tensor_scalar(agg2[:], agg2[:], 0.0, None, op0=AOP.max)
    g1b_b = c_g1w[:, None, :].broadcast_to([128, NB, 128])
    g2b_b = c_g2w[:, None, :].broadcast_to([128, NB, 128])
    DV.drain()
    DV.tensor_tensor(tmp[:], agg1[:], g1b_b, op=AOP.mult)
    DV.drain()
    DV.tensor_reduce(lamv["l1"][:], tmp[:], axis=mybir.AxisListType.X, op=AOP.add)
    DV.drain()
    DV.tensor_tensor(tmp[:], agg2[:], g2b_b, op=AOP.mult)
    DV.drain()
    dv.inc(DV.tensor_reduce(lamv["l2"][:], tmp[:], axis=mybir.AxisListType.X,
                            op=AOP.add))
    AC.wait_ge(dv.sem, dv.n)
    AC.activation(lamv["l1"][:], lamv["l1"][:], ACT.Sigmoid, bias=cbias[:, 0:1])
    ac.inc(AC.activation(lamv["l2"][:], lamv["l2"][:], ACT.Sigmoid, bias=cbias[:, 1:2]))
    DV.wait_ge(ac.sem, ac.n)
    DV.tensor_tensor(lamv["lsum"][:], lamv["l1"][:], lamv["l2"][:], op=AOP.add)
    DV.drain()
    DV.tensor_scalar(lamv["lsum"][:], lamv["lsum"][:], 1e-12, None, op0=AOP.max)
    DV.drain()
    DV.reciprocal(lamv["lsum"][:], lamv["lsum"][:])
    DV.drain()
    DV.tensor_tensor(lamv["w0"][:], lamv["l1"][:], lamv["lsum"][:], op=AOP.mult)
    DV.tensor_tensor(lamv["w1"][:], lamv["l2"][:], lamv["lsum"][:], op=AOP.mult)
    w0b = lamv["w0"][:, :, None].broadcast_to([128, NB, 128])
    w1b_ = lamv["w1"][:, :, None].broadcast_to([128, NB, 128])
    DV.drain()
    DV.tensor_tensor(agg1[:], agg1[:], w0b, op=AOP.mult)
    DV.tensor_tensor(agg2[:], agg2[:], w1b_, op=AOP.mult)
    DV.drain()
    DV.tensor_tensor(agg1[:], agg1[:], agg2[:], op=AOP.add)   # x -> agg1
    DV.drain()
    dv.inc(DV.memset(tstage[:], 0))
    xfin = dv.n

    # L2 table: s2 = x @ W2 (pad to 64 cols), rows bf16-padded to 128
    s2_ps = tr_ps  # reuse [128,128] psum tiles
    stc = {}
    for t in range(NB):
        p = s2_ps[t % 2]
        if t == 0:
            PE.wait_ge(dv.sem, xfin)
        if t >= 2:
            PE.wait_ge(dv.sem, stc[t - 2])
        pe.inc(PE.transpose(p[:], agg1[:, t, :], c_idf[:]))
        DV.wait_ge(pe.sem, pe.n)
        dv.inc(DV.tensor_copy(xtt[:], p[:]))
        PE.wait_ge(dv.sem, dv.n)
        pe.inc(PE.matmul(p[:, 0:64], xtt[:], c_w2[:], start=True, stop=True))
        DV.wait_ge(pe.sem, pe.n)
        dv.inc(DV.tensor_copy(tstage[:, t, 0:64], p[:, 0:64]))
        stc[t] = dv.n
    SY.wait_ge(dv.sem, dv.n)
    io.inc(SY.dma_start(t_in[3][:].rearrange("(t p) f -> p t f", p=128), tstage[:]))
    fence()
    GP.wait_ge(io.sem, io.n)
    ccs[2].inc(GP.collective_compute(
        "AllGather", AOP.bypass, replica_groups=[list(range(NCORES))],
        ins=[t_in[3][:]], outs=[t_full[3][:]]))

    # =========== L2 edge passes ===========
    edge_pass(1, t_full[3], 64, prop1, 2, m1[0], m1[1], m1[3], m1[4], False)
    edge_pass(2, t_full[3], 64, prop2, 2, m2[0], m2[1], m2[3], m2[4], False)

    # =========== Phase F: final gating + outputs ===========
    DV.drain()
    b2b = c_b2r[:, None, :].broadcast_to([128, NB, 64])
    DV.tensor_tensor(prop1[:], prop1[:], b2b, op=AOP.add)
    DV.drain()
    dv.inc(DV.tensor_tensor(prop2[:], prop2[:], b2b, op=AOP.add))
    pfin = dv.n
    SY.wait_ge(dv.sem, pfin)
    io.inc(SY.dma_start(p1_o[:].rearrange("(t p) f -> p t f", p=128),
                        prop1[:, :, 0:NCLASS]))
    io.inc(SY.dma_start(p2_o[:].rearrange("(t p) f -> p t f", p=128),
                        prop2[:, :, 0:NCLASS]))
    fence()
    pout_io = io.n
    h1b_b = c_h1w[:, None, :].broadcast_to([128, NB, 64])
    h2b_b = c_h2w[:, None, :].broadcast_to([128, NB, 64])
    t64 = tmp[:, :, 0:64]
    DV.tensor_tensor(t64, prop1[:], h1b_b, op=AOP.mult)
    DV.drain()
    DV.tensor_reduce(lamv["l1"][:], t64, axis=mybir.AxisListType.X, op=AOP.add)
    DV.drain()
    DV.tensor_tensor(t64, prop2[:], h2b_b, op=AOP.mult)
    DV.drain()
    dv.inc(DV.tensor_reduce(lamv["l2"][:], t64, axis=mybir.AxisListType.X,
                            op=AOP.add))
    AC.wait_ge(dv.sem, dv.n)
    AC.activation(lamv["l1"][:], lamv["l1"][:], ACT.Sigmoid, bias=cbias[:, 2:3])
    ac.inc(AC.activation(lamv["l2"][:], lamv["l2"][:], ACT.Sigmoid, bias=cbias[:, 3:4]))
    DV.wait_ge(ac.sem, ac.n)
    DV.tensor_tensor(lamv["lsum"][:], lamv["l1"][:], lamv["l2"][:], op=AOP.add)
    DV.drain()
    DV.tensor_scalar(lamv["lsum"][:], lamv["lsum"][:], 1e-12, None, op0=AOP.max)
    DV.drain()
    DV.reciprocal(lamv["lsum"][:], lamv["lsum"][:])
    DV.drain()
    DV.tensor_tensor(lamv["w0"][:], lamv["l1"][:], lamv["lsum"][:], op=AOP.mult)
    DV.tensor_tensor(lamv["w1"][:], lamv["l2"][:], lamv["lsum"][:], op=AOP.mult)
    DV.drain()
    w0b6 = lamv["w0"][:, :, None].broadcast_to([128, NB, 64])
    w1b6 = lamv["w1"][:, :, None].broadcast_to([128, NB, 64])
    DV.wait_ge(io.sem, pout_io)  # don't clobber props mid-DMA
    DV.tensor_tensor(t64, prop1[:], w0b6, op=AOP.mult)
    DV.tensor_tensor(prop2[:], prop2[:], w1b6, op=AOP.mult)
    DV.drain()
    dv.inc(DV.tensor_tensor(t64, t64, prop2[:], op=AOP.add))
    SY.wait_ge(dv.sem, dv.n)
    io.inc(SY.dma_start(out_o[:].rearrange("(t p) f -> p t f", p=128),
                        tmp[:, :, 0:NCLASS]))
    SY.wait_ge(io.sem, io.n)

    nc.compile()
    ctx.close()
    return nc


def _run(inputs, sim=False):
    S = inputs["x1a"].shape[0] // NCORES
    NB = -(-S // 128)
    SP = NB * 128
    NROWS = NCORES * SP
    HSPLIT = min(32768, NROWS // 2 // 128 * 128)

    adj = {}
    adjmeta = {}
    for a in (1, 2):
        out, cpb_lo, cpb_hi, ch_lo, ch_hi, nslot = _prep_adjacency(
            inputs[f"src{a}"], inputs[f"dst{a}"], inputs[f"ew{a}"],
            S, SP, NB, HSPLIT, NROWS)
        adj[a] = out
        adjmeta[a] = (ch_lo, ch_hi, nslot, cpb_lo, cpb_hi)

    scalars = (float(np.asarray(inputs["g1b"]).ravel()[0]),
               float(np.asarray(inputs["g2b"]).ravel()[0]),
               float(np.asarray(inputs["h1b"]).ravel()[0]),
               float(np.asarray(inputs["h2b"]).ravel()[0]))
    nc = _build(S, SP, NB, NROWS, HSPLIT, adjmeta, scalars)

    bf = ml_dtypes.bfloat16
    f32 = np.float32

    def wfmt(w):  # [256, 64] -> [128, 2, 64] bf16
        return np.ascontiguousarray(
            np.asarray(w, f32).reshape(2, 128, NHID).transpose(1, 0, 2)).astype(bf)

    w2pad = np.zeros((128, 64), f32)
    w2pad[:, :NCLASS] = np.asarray(inputs["W2"], f32)
    iota = np.tile(np.arange(128, dtype=f32), (128, 1))
    ident = np.eye(128, dtype=f32)
    g1w = np.tile(np.asarray(inputs["g1w"], f32).ravel(), (128, 1))
    g2w = np.tile(np.asarray(inputs["g2w"], f32).ravel(), (128, 1))
    h1w = np.zeros((128, 64), f32)
    h1w[:, :NCLASS] = np.asarray(inputs["h1w"], f32).ravel()
    h2w = np.zeros((128, 64), f32)
    h2w[:, :NCLASS] = np.asarray(inputs["h2w"], f32).ravel()
    b1r = np.tile(np.concatenate([np.asarray(inputs["b1a"], f32).ravel(),
                                  np.asarray(inputs["b1b"], f32).ravel()]), (128, 1))
    b2r = np.zeros((128, 64), f32)
    b2r[:, :NCLASS] = np.asarray(inputs["b2"], f32).ravel()

    common = dict(
        w1a=wfmt(inputs["W1a"]), w1b=wfmt(inputs["W1b"]),
        w2=w2pad.astype(bf), iota=iota.astype(bf), idf=ident,
        idb=ident.astype(bf), g1w=g1w, g2w=g2w, h1w=h1w, h2w=h2w,
        b1r=b1r, b2r=b2r)

    def xfmt(x, k):  # shard k, pad, transpose -> [128, 2, SP] bf16
        xs = np.asarray(x, f32)[k * S:(k + 1) * S]
        xp = np.zeros((SP, NFEAT), f32)
        xp[:S] = xs
        xt = xp.T.reshape(2, 128, SP).transpose(1, 0, 2)
        return np.ascontiguousarray(xt).astype(bf)

    in_maps = []
    for k in range(NCORES):
        m = dict(common)
        for v, key in (("xt1a", "x1a"), ("xt1b", "x1b"),
                       ("xt2a", "x2a"), ("xt2b", "x2b")):
            m[v] = xfmt(inputs[key], k)
        for a in (1, 2):
            g, d, e = adj[a][k]
            m[f"gidx{a}"] = g
            m[f"dst{a}"] = d
            m[f"eww{a}"] = e
        in_maps.append(m)

    global LAST_EXEC_NS
    if sim:
        from concourse.bass_interp import MultiCoreSim
        msim = MultiCoreSim(nc, NCORES)
        for k in range(NCORES):
            for name, arr in in_maps[k].items():
                msim.cores[k].tensor(name)[:] = arr
        msim.simulate()
        results = [{nm: msim.cores[k].tensor(nm).copy()
                    for nm in ("out_o", "p1_o", "p2_o")} for k in range(NCORES)]
    else:
        import os
        import time as _time
        trace = bool(os.environ.get("KERNEL_TRACE"))
        r = run_bass_kernel_spmd(nc, in_maps, list(range(NCORES)), trace=trace)
        LAST_EXEC_NS = r.exec_time_ns
        results = r.results
        if os.environ.get("KERNEL_REPEAT"):
            t0 = _time.perf_counter()
            run_bass_kernel_spmd(nc, in_maps, list(range(NCORES)))
            global LAST_WALL2_S
            LAST_WALL2_S = _time.perf_counter() - t0

    outs = []
    for nm in ("out_o", "p1_o", "p2_o"):
        outs.append(np.concatenate([results[k][nm][:S] for k in range(NCORES)],
                    axis=0).astype(np.float32))
    return tuple(outs)


LAST_EXEC_NS = None
LAST_WALL2_S = None


def kernel(**inputs):
    return _run(inputs, sim=False)



# revision 27
# speedup vs baseline: 1.3340x; 1.3340x over previous
"""DaGCN on 8 Trainium2 NeuronCores (Bass SPMD).

Strategy (graph/data parallel, nodes sharded 8 ways):
  * Each core owns a 6250-node shard (padded to 6272 = 49*128).
  * L1 feature tables s = [x_a@W1a | x_b@W1b] for both views are staged as
    interleaved 512B rows ([SP, 256] bf16: view1 cols 0:128, view2 128:256)
    and exchanged with ONE batched AllGather issued on the SP engine (so it
    never blocks the Pool engine's gather stream).
  * Edges are assigned to the core owning dst, laid out in call-aligned
    chunk segments per (block-range, src-row window). dma_gather
    (1024 idxs/call, elem_step=256 to skip the interleaved other view)
    fetches s[src] as bf16 messages.
  * segment_sum runs on the TensorEngine: per 128-edge chunk a one-hot
    lhsT [128 edges x 128 dst-cols] holding ew (built on DVE from an iota
    compare) is matmul'ed with the message chunk, accumulating each dst
    block in PSUM; drains add the layer bias.
  * Gating is pipelined per block behind the edge passes: relu + fused
    multiply-reduce (tensor_tensor_reduce) right after each block's final
    drain; per-group sigmoid/normalize/combine + the L2 table build
    (transpose + matmul with W2).
  * The L2 table y = x@W2 is exchanged COMPACT ([*,32] bf16) in two
    AllGather halves (blocks 0:25 launched mid-pass, 25:49 at pass end),
    then expanded locally into 256B gather rows with a cheap strided DMA.
"""

import math
from contextlib import ExitStack

import ml_dtypes
import numpy as np

import concourse.bacc as bacc
import concourse.bass as bass
import concourse.mybir as mybir
from concourse.bass_utils import run_bass_kernel_spmd

F32 = mybir.dt.float32
BF16 = mybir.dt.bfloat16
I16 = mybir.dt.int16
AOP = mybir.AluOpType
ACT = mybir.ActivationFunctionType

NCORES = 8
N = 50000
NFEAT, NHID, NCLASS = 256, 64, 32
S = N // NCORES                 # 6250
NB = -(-S // 128)               # 49
SP = NB * 128                   # 6272
NROWS = NCORES * SP             # 50176
HSPLIT = NROWS // 2 // 128 * 128  # 25088 (int16 window split for L1 table)
B1 = 25                         # block split for the two L2-table collectives
A_ROWS = B1 * 128               # 3200 rows/core in table A
B_ROWS = SP - A_ROWS            # 3072 rows/core in table B
S_CALL = 1024                   # idxs per dma_gather call
CALL_CHUNKS = S_CALL // 128
RING = 12                       # gather/onehot ring depth (in calls)
NPSUM = 4                       # psum block-accumulator ring


def _wrap16(a):
    """[n] int16 -> [128, n//16]: idx i at [i%16, i//16], replicated x8."""
    n = a.shape[0]
    w = a.reshape(n // 16, 16).T.astype(np.int16)
    return np.tile(w, (8, 1)).copy()


def _chunkwrap(a, dtype):
    """[n] -> [128, n//128]: edge i at [i%128, i//128]."""
    n = a.shape[0]
    return np.ascontiguousarray(a.reshape(n // 128, 128).T.astype(dtype))


def _segments_l1_p1():
    return [(0, NB, 0), (0, NB, 1)]


def _segments_l1_p2():
    return [(0, B1, 0), (0, B1, 1), (B1, NB, 0), (B1, NB, 1)]


def _segments_l2():
    return [(0, NB, 0), (0, NB, 1)]


def _layout(segments, cpb):
    """Chunk schedule: per chunk (block, start, stop); per segment call count.

    Pad chunks (call alignment) are folded into the segment's last block
    with ew=0 so they accumulate zero into its psum chain.
    """
    chunks = []           # (seg_idx, block, start, stop)
    seg_calls = []
    for si, (b0, b1, w) in enumerate(segments):
        nblk = b1 - b0
        nch_raw = nblk * cpb[w]
        nch = -(-nch_raw // CALL_CHUNKS) * CALL_CHUNKS
        npad = nch - nch_raw
        for c in range(nch):
            if c < nch_raw:
                b = b0 + c // cpb[w]
                st = (c % cpb[w] == 0)
                sp_ = (c % cpb[w] == cpb[w] - 1) and (b < b1 - 1)
            else:
                b = b1 - 1
                st = False
                sp_ = False
            # last block's stop is the final chunk of the segment
            if c == nch - 1:
                sp_ = True
            chunks.append((si, b, st, sp_))
        seg_calls.append(nch // CALL_CHUNKS)
    return chunks, seg_calls


def _prep_pass(src, dst, ew, window_of, localrow_of, segments):
    """Slot arrays for one (adjacency, layer) pass.

    Returns per-core (gidx_wrapped, dcol, eww), cpb per window, layout.
    """
    src = np.asarray(src).astype(np.int64)
    dst = np.asarray(dst).astype(np.int64)
    ew = np.asarray(ew).astype(np.float32)
    core = dst // S
    dstrel = dst - core * S
    blk = dstrel // 128
    col = dstrel % 128
    win = window_of(src)
    lrow = localrow_of(src)

    counts = np.zeros((NCORES, 2, NB), np.int64)
    percore = []
    for k in range(NCORES):
        m = core == k
        e = np.lexsort((blk[m], win[m]))
        r, w_, b, c, wt = lrow[m][e], win[m][e], blk[m][e], col[m][e], ew[m][e]
        percore.append((r, w_, b, c, wt))
        for hh in range(2):
            mm = w_ == hh
            counts[k, hh] = np.bincount(b[mm], minlength=NB)
    cpb = [max(1, int(np.ceil(counts[:, h].max() / 128))) for h in range(2)]

    chunks, seg_calls = _layout(segments, cpb)
    nslot = len(chunks) * 128
    # slot offset of each (segment, block) run
    run_of = {}
    for ci, (si, b, st, sp_) in enumerate(chunks):
        if st or (si, b) not in run_of:
            if (si, b) not in run_of:
                run_of[(si, b)] = ci * 128

    out = []
    for k in range(NCORES):
        r, w_, b, c, wt = percore[k]
        gidx = np.zeros(nslot, np.int64)
        dcol = np.zeros(nslot, np.int64)
        eww = np.zeros(nslot, np.float32)
        for si, (b0, b1, wseg) in enumerate(segments):
            mm = (w_ == wseg) & (b >= b0) & (b < b1)
            rr, bb, cc, ww = r[mm], b[mm], c[mm], wt[mm]
            cnt = counts[k, wseg]
            offs = np.zeros(NB, np.int64)
            offs[b0:b1] = np.concatenate(
                ([0], np.cumsum(cnt[b0:b1])))[:-1]
            pos = np.arange(rr.shape[0]) - offs[bb]
            slot = run_of[(si, b0)] + (bb - b0) * cpb[wseg] * 128 + pos
            gidx[slot] = rr
            dcol[slot] = cc
            eww[slot] = ww
        out.append((
            _wrap16(gidx),
            _chunkwrap(dcol, np.float32),
            _chunkwrap(eww, np.float32),
        ))
    return out, cpb, chunks, seg_calls, nslot


def _prep_all(inputs):
    """Host-side edge prep for all four passes."""

    def l1_window(s):
        grow = (s // S) * SP + (s % S)
        return (grow >= HSPLIT).astype(np.int64)

    def l1_lrow(s):
        grow = (s // S) * SP + (s % S)
        return grow - (grow >= HSPLIT) * HSPLIT

    def l2_window(s):
        return ((s % S) >= A_ROWS).astype(np.int64)

    def l2_lrow(s):
        c = s // S
        slot = s % S
        hi = slot >= A_ROWS
        return np.where(hi, c * B_ROWS + (slot - A_ROWS), c * A_ROWS + slot)

    prep = {}
    segs = {
        ("l1", 1): _segments_l1_p1(),
        ("l1", 2): _segments_l1_p2(),
        ("l2", 1): _segments_l2(),
        ("l2", 2): _segments_l2(),
    }
    for a in (1, 2):
        sa, da, wa = inputs[f"src{a}"], inputs[f"dst{a}"], inputs[f"ew{a}"]
        prep[("l1", a)] = _prep_pass(sa, da, wa, l1_window, l1_lrow,
                                     segs[("l1", a)])
        prep[("l2", a)] = _prep_pass(sa, da, wa, l2_window, l2_lrow,
                                     segs[("l2", a)])
    return prep, segs


class Ctr:
    def __init__(self, sem, step=1):
        self.sem, self.n, self.step = sem, 0, step

    def inc(self, inst):
        inst.then_inc(self.sem, self.step)
        self.n += self.step
        return self.n


def _build(prep, segs, scalars):
    nc = bacc.Bacc("TRN2", num_devices=NCORES, num_swdge_queues=2)
    g1b, g2b, h1b, h2b = scalars

    meta = {}  # key -> (cpb, chunks, seg_calls, nslot)
    for key, (out, cpb, chunks, seg_calls, nslot) in prep.items():
        meta[key] = (cpb, chunks, seg_calls, nslot)
    nslot_l1 = max(meta[("l1", a)][3] for a in (1, 2))
    nslot_l2 = max(meta[("l2", a)][3] for a in (1, 2))
    nslot_max = max(nslot_l1, nslot_l2)
    nch_max = nslot_max // 128

    # ---------------- I/O ----------------
    din = {}
    for v in ("xt1a", "xt1b", "xt2a", "xt2b"):
        din[v] = nc.dram_tensor(v, [128, 2, SP], BF16, kind="ExternalInput")
    din["w1a"] = nc.dram_tensor("w1a", [128, 2, NHID], BF16, kind="ExternalInput")
    din["w1b"] = nc.dram_tensor("w1b", [128, 2, NHID], BF16, kind="ExternalInput")
    din["w2"] = nc.dram_tensor("w2", [128, NCLASS], BF16, kind="ExternalInput")
    din["iota"] = nc.dram_tensor("iota", [128, 128], BF16, kind="ExternalInput")
    din["idb"] = nc.dram_tensor("idb", [128, 128], BF16, kind="ExternalInput")
    din["g1w"] = nc.dram_tensor("g1w", [128, 1], BF16, kind="ExternalInput")
    din["g2w"] = nc.dram_tensor("g2w", [128, 1], BF16, kind="ExternalInput")
    din["h1w"] = nc.dram_tensor("h1w", [128, NCLASS], BF16, kind="ExternalInput")
    din["h2w"] = nc.dram_tensor("h2w", [128, NCLASS], BF16, kind="ExternalInput")
    din["b1c"] = nc.dram_tensor("b1c", [128, 1], F32, kind="ExternalInput")
    din["b2r"] = nc.dram_tensor("b2r", [128, NCLASS], F32, kind="ExternalInput")
    for lay in ("l1", "l2"):
        for a in (1, 2):
            ns = meta[(lay, a)][3]
            din[f"gidx_{lay}{a}"] = nc.dram_tensor(
                f"gidx_{lay}{a}", [128, ns // 16], I16, kind="ExternalInput")
            din[f"dst_{lay}{a}"] = nc.dram_tensor(
                f"dst_{lay}{a}", [128, ns // 128], F32, kind="ExternalInput")
            din[f"eww_{lay}{a}"] = nc.dram_tensor(
                f"eww_{lay}{a}", [128, ns // 128], F32, kind="ExternalInput")
    out_o = nc.dram_tensor("out_o", [SP, NCLASS], F32, kind="ExternalOutput")
    p1_o = nc.dram_tensor("p1_o", [SP, NCLASS], F32, kind="ExternalOutput")
    p2_o = nc.dram_tensor("p2_o", [SP, NCLASS], F32, kind="ExternalOutput")

    t12_in = nc.dram_tensor("t12_in", [SP, 256], BF16)
    t12_full = nc.dram_tensor("t12_full", [NROWS, 256], BF16, addr_space="Shared")
    t3as_in = nc.dram_tensor("t3as_in", [A_ROWS, NCLASS], BF16)
    t3bs_in = nc.dram_tensor("t3bs_in", [B_ROWS, NCLASS], BF16)
    t3as_full = nc.dram_tensor("t3as_full", [NCORES * A_ROWS, NCLASS], BF16,
                               addr_space="Shared")
    t3bs_full = nc.dram_tensor("t3bs_full", [NCORES * B_ROWS, NCLASS], BF16,
                               addr_space="Shared")
    t3a_full = nc.dram_tensor("t3a_full", [NCORES * A_ROWS, 128], BF16)
    t3b_full = nc.dram_tensor("t3b_full", [NCORES * B_ROWS, 128], BF16)

    ctx = ExitStack()
    sb = lambda name, shape, dt: ctx.enter_context(nc.sbuf_tensor(name, shape, dt))
    ps = lambda name, shape, dt=F32: ctx.enter_context(nc.psum_tensor(name, shape, dt))
    sem = lambda name: ctx.enter_context(nc.semaphore(name))

    # ---------------- SBUF: constants ----------------
    c_w1a = sb("c_w1a", [128, 2, NHID], BF16)
    c_w1b = sb("c_w1b", [128, 2, NHID], BF16)
    c_w2 = sb("c_w2", [128, NCLASS], BF16)
    c_iota = sb("c_iota", [128, 128], BF16)
    c_idb = sb("c_idb", [128, 128], BF16)
    c_g1w = sb("c_g1w", [128, 1], BF16)
    c_g2w = sb("c_g2w", [128, 1], BF16)
    c_h1w = sb("c_h1w", [128, NCLASS], BF16)
    c_h2w = sb("c_h2w", [128, NCLASS], BF16)
    c_b1c = sb("c_b1c", [128, 1], F32)
    c_b2r = sb("c_b2r", [128, NCLASS], F32)
    cbias = sb("cbias", [128, 4], F32)

    # ---------------- SBUF: state ----------------
    agg1 = sb("agg1", [128, NB, 128], BF16)
    agg2 = sb("agg2", [128, NB, 128], BF16)
    tmp = sb("tmp", [128, NB, NCLASS], BF16)    # y-combine scratch
    prop1 = sb("prop1", [128, NB, NCLASS], F32)
    prop2 = sb("prop2", [128, NB, NCLASS], F32)
    outbuf = sb("outbuf", [128, NB, NCLASS], F32)
    outscr = sb("outscr", [128, NB, NCLASS], F32)
    ystage = sb("ystage", [128, NB, NCLASS], BF16)
    lam = {nm: sb("lam_" + nm, [128, NB], F32)
           for nm in ("l1", "l2", "ls", "w0", "w1")}

    # idx buffers: two resident sets; set 0 = L1a then L2b, set 1 = L1b.
    # L2a gets its own buffers carved from the phase-A x region (see below).
    gidx_sb = [sb(f"gidx_sb{i}", [128, nslot_max // 16], I16) for i in range(2)]
    dst_sb = [sb(f"dst_sb{i}", [128, nch_max], F32) for i in range(2)]
    ew_sb = [sb(f"ew_sb{i}", [128, nch_max], F32) for i in range(2)]

    # phase-A region (freed for rings + L2a idx)
    sbA = ExitStack()
    sT = sbA.enter_context(nc.sbuf_tensor("sT", [128, SP], BF16))
    t12st = sbA.enter_context(nc.sbuf_tensor("t12st", [128, NB, 128], BF16))
    xta = sbA.enter_context(nc.sbuf_tensor("xta", [128, 2, SP], BF16))
    xtb = sbA.enter_context(nc.sbuf_tensor("xtb", [128, 2, SP], BF16))

    psA = ExitStack()
    mm_ps = [psA.enter_context(nc.psum_tensor(f"mm_ps{i}", [128, 512], F32))
             for i in range(2)]
    trA_ps = [psA.enter_context(nc.psum_tensor(f"trA_ps{i}", [128, 128], BF16))
              for i in range(2)]

    io = Ctr(sem("io"), 16)        # SP-engine DMAs
    io2 = Ctr(sem("io2"), 16)      # ACT-engine DMAs
    cc = Ctr(sem("cc"), 1)         # collectives (SP engine)
    gsems = [Ctr(sem(f"g{i}"), 16) for i in range(RING)]
    pe = Ctr(sem("pe"), 1)
    dv = Ctr(sem("dv"), 1)
    ac = Ctr(sem("ac"), 1)

    SY, PE, DV, AC, GP = nc.sync, nc.tensor, nc.vector, nc.scalar, nc.gpsimd

    def fence_sp():
        SY.wait_ge(io.sem, io.n)

    def fence_ac():
        AC.wait_ge(io2.sem, io2.n)

    def gp_allgather(in_ap, out_ap):
        return GP.collective_compute(
            "AllGather", AOP.bypass, replica_groups=[list(range(NCORES))],
            ins=[in_ap], outs=[out_ap])

    # =========== constants + idx preloads ===========
    for bi, bval in enumerate((g1b, g2b, h1b, h2b)):
        dv.inc(DV.memset(cbias[:, bi:bi + 1], float(bval)))
    for name, t in (("w1a", c_w1a), ("w1b", c_w1b), ("w2", c_w2),
                    ("iota", c_iota), ("idb", c_idb), ("g1w", c_g1w),
                    ("g2w", c_g2w), ("h1w", c_h1w), ("h2w", c_h2w),
                    ("b1c", c_b1c), ("b2r", c_b2r)):
        io.inc(SY.dma_start(t[:], din[name][:]))
    consts_io = io.n
    fence_sp()

    idx_io = {}

    # =========== Phase A: s tables ===========
    nsl = [(j * 512, min(512, SP - j * 512)) for j in range((SP + 511) // 512)]

    def s_table2(vcol, va, vb, wa, wb, tst_ready_dv):
        io.inc(SY.dma_start(xta[:], din[va][:]))
        io2.inc(AC.dma_start(xtb[:], din[vb][:]))
        xload = io.n
        xload2 = io2.n
        fence_sp()
        fence_ac()
        copies = []
        for j, (o, n) in enumerate(nsl):
            p = mm_ps[j % 2]
            if j == 0:
                PE.wait_ge(io.sem, xload)
                PE.wait_ge(io2.sem, xload2)
            if j >= 2:
                PE.wait_ge(dv.sem, copies[j - 2])
            for xt, w, prow in ((xta, wa, 0), (xtb, wb, 64)):
                for cch in range(2):
                    last = PE.matmul(p[prow:prow + 64, 0:n], w[:, cch, :],
                                     xt[:, cch, o:o + n],
                                     start=(cch == 0), stop=(cch == 1))
            pe.inc(last)
            DV.wait_ge(pe.sem, pe.n)
            cp = DV.tensor_copy(sT[:, o:o + n], p[:, 0:n])
            dv.inc(cp)
            copies.append(dv.n)
        # transpose into t12st
        trc = {}
        for t in range(NB):
            p = trA_ps[t % 2]
            PE.wait_ge(dv.sem, copies[-1])
            if t == 0 and tst_ready_dv is not None:
                # t12st reuse across tables: wait for the first write-out
                DV.wait_ge(io.sem, tst_ready_dv)
            if t >= 2:
                PE.wait_ge(dv.sem, trc[t - 2])
            pe.inc(PE.transpose(p[:], sT[:, t * 128:(t + 1) * 128], c_idb[:]))
            DV.wait_ge(pe.sem, pe.n)
            dv.inc(DV.tensor_copy(t12st[:, t, :], p[:]))
            trc[t] = dv.n
        SY.wait_ge(dv.sem, dv.n)
        dst_ap = t12_in[:, vcol * 128:(vcol + 1) * 128]
        io.inc(SY.dma_start(dst_ap.rearrange("(t p) f -> p t f", p=128),
                            t12st[:]))
        fence_sp()
        return io.n, pe.n

    t1_io, t1_pe = s_table2(0, "xt1a", "xt1b", c_w1a, c_w1b, None)
    SY.wait_ge(pe.sem, t1_pe)          # xta reuse by table 2
    AC.wait_ge(pe.sem, t1_pe)          # xtb likewise (ACT-issued load)
    t2_io, t2_pe = s_table2(1, "xt2a", "xt2b", c_w1a, c_w1b, t1_io)
    pe_phaseA = pe.n

    # batched AllGather of both L1 tables (Pool engine)
    GP.wait_ge(io.sem, io.n)
    cc.inc(gp_allgather(t12_in[:], t12_full[:]))
    cc01 = cc.n                         # ==1

    # L1 idx preloads (ACT engine; in-order after the xtb loads)
    for slot, key in ((0, ("l1", 1)), (1, ("l1", 2))):
        ns = meta[key][3]
        lay, a = key
        io2.inc(AC.dma_start(gidx_sb[slot][:, 0:ns // 16], din[f"gidx_{lay}{a}"][:]))
        io2.inc(AC.dma_start(dst_sb[slot][:, 0:ns // 128], din[f"dst_{lay}{a}"][:]))
        io2.inc(AC.dma_start(ew_sb[slot][:, 0:ns // 128], din[f"eww_{lay}{a}"][:]))
        idx_io[key] = io2.n
        fence_ac()

    # free phase-A SBUF/PSUM; reuse for rings + L2a idx
    psA.close()
    sbA.close()
    blk_ps = [ps(f"blk_ps{i}", [128, 128]) for i in range(NPSUM)]
    y_ps = [ps(f"y_ps{i}", [128, 2 * NCLASS]) for i in range(2)]
    lam_ps = ps("lam_ps", [128, 4])
    msg = sb("msg", [128, RING * CALL_CHUNKS, 128], BF16)
    ohr = sb("ohr", [128, RING * CALL_CHUNKS, 128], BF16)
    gidx_l2a = sb("gidx_l2a", [128, nslot_l2 // 16], I16)
    dst_l2a = sb("dst_l2a", [128, nslot_l2 // 128], F32)
    ew_l2a = sb("ew_l2a", [128, nslot_l2 // 128], F32)

    # L2a idx loads (ACT; region aliases freed xta/xtb -> wait phase-A PE)
    AC.wait_ge(pe.sem, pe_phaseA)
    ns = meta[("l2", 1)][3]
    io2.inc(AC.dma_start(gidx_l2a[:, 0:ns // 16], din["gidx_l21"][:]))
    io2.inc(AC.dma_start(dst_l2a[:, 0:ns // 128], din["dst_l21"][:]))
    io2.inc(AC.dma_start(ew_l2a[:, 0:ns // 128], din["eww_l21"][:]))
    idx_io[("l2", 1)] = io2.n
    fence_ac()

    # L1 table window APs (elem_step=256 skips the other view's 256B half)
    l1_tabs = {
        1: [t12_full[0:HSPLIT, 0:128], t12_full[HSPLIT:NROWS, 0:128]],
        2: [t12_full[0:HSPLIT, 128:256], t12_full[HSPLIT:NROWS, 128:256]],
    }
    l2_tabs = [t3a_full[:], t3b_full[:]]

    # =========== edge pass machinery ===========
    gcall = [0]
    pe_cons_vals = []
    psum_last = [0] * NPSUM            # dv value of drain freeing each slot
    first_gather = [True]
    first_onehot = [True]
    segs_by_key = segs

    def edge_pass(key, bufs, tabs, elem_step, F_rhs, dest, cbias_row,
                  cc_need_by_w, io_need_by_w, block_tail, seg_end_hook,
                  drain_seg_of_block, seg_range=None, first_drained=None,
                  transposed=False):
        """One pass. bufs = (gidx, dst, ew) SBUF APs.
        cc_need_by_w / io_need_by_w: per-window sem thresholds.
        block_tail(b): emit per-block tail after b's final drain.
        seg_end_hook(si): emit after segment si's last call is consumed.
        drain_seg_of_block: {b: seg_idx of b's LAST segment}.
        seg_range: restrict to these segment indices (window-major L2 order).
        """
        cpb, chunks, seg_calls, nslot = meta[key]
        gix, dsb, ewb = bufs
        idxv = idx_io[key]
        if first_drained is None:
            first_drained = set()
        for si, ncalls in enumerate(seg_calls):
            call0 = sum(seg_calls[:si])
            if seg_range is not None and si not in seg_range:
                continue
            b0, b1_, w = (segs_by_key[key])[si]
            for j in range(ncalls):
                call = call0 + j
                rj = (gcall[0] % RING) * CALL_CHUNKS
                gslot = gcall[0] % RING
                # ---- gather ----
                GP.wait_ge(io2.sem, idxv)
                GP.wait_ge(cc.sem, cc_need_by_w[w])
                if io_need_by_w[w]:
                    GP.wait_ge(io.sem, io_need_by_w[w])
                if first_gather[0]:
                    GP.wait_ge(pe.sem, pe_phaseA)   # msg aliases freed xta
                    GP.wait_ge(io.sem, t2_io)       # and staged sT/t12st
                    first_gather[0] = False
                if len(pe_cons_vals) >= RING:
                    GP.wait_ge(pe.sem, pe_cons_vals[-RING])
                g = GP.dma_gather(
                    msg[:, rj:rj + CALL_CHUNKS, :], tabs[w],
                    gix[:, call * (S_CALL // 16):(call + 1) * (S_CALL // 16)],
                    S_CALL, S_CALL, 128, elem_step=elem_step,
                    queue_num=gcall[0] % 2)
                gsems[gslot].inc(g)
                gv = gsems[gslot].n
                # ---- onehot ----
                DV.wait_ge(io2.sem, idxv)
                if first_onehot[0]:
                    DV.wait_ge(pe.sem, pe_phaseA)   # ohr aliases phase-A sbuf
                    DV.wait_ge(io.sem, t2_io)
                    first_onehot[0] = False
                if len(pe_cons_vals) >= RING:
                    DV.wait_ge(pe.sem, pe_cons_vals[-RING])
                cbase = call * CALL_CHUNKS
                for c8 in range(CALL_CHUNKS):
                    ts = DV.tensor_scalar(
                        ohr[:, rj + c8, :], c_iota[:],
                        dsb[:, cbase + c8:cbase + c8 + 1],
                        ewb[:, cbase + c8:cbase + c8 + 1],
                        op0=AOP.is_equal, op1=AOP.mult)
                dv.inc(ts)
                ohv = dv.n
                # ---- matmuls ----
                PE.wait_ge(gsems[gslot].sem, gv)
                PE.wait_ge(dv.sem, ohv)
                for c8 in range(CALL_CHUNKS):
                    ci = cbase + c8
                    _, b, st, sp_ = chunks[ci]
                    slot = b % NPSUM
                    p = blk_ps[slot]
                    if st and psum_last[slot]:
                        PE.wait_ge(dv.sem, psum_last[slot])
                    if transposed:
                        # psum accumulates [feat, dstcol]
                        mmi = PE.matmul(p[:, 0:128],
                                        msg[:, rj + c8, 0:F_rhs],
                                        ohr[:, rj + c8, :],
                                        start=st, stop=sp_)
                    else:
                        mmi = PE.matmul(p[:, 0:F_rhs],
                                        ohr[:, rj + c8, :],
                                        msg[:, rj + c8, 0:F_rhs],
                                        start=st, stop=sp_)
                    if sp_:
                        pe.inc(mmi)
                        # ---- drain ----
                        DV.wait_ge(pe.sem, pe.n)
                        ncol = 128 if transposed else F_rhs
                        if b not in first_drained:
                            if cbias_row is None:
                                d = DV.tensor_copy(dest[:, b, 0:ncol],
                                                   p[:, 0:ncol])
                            else:
                                d = DV.tensor_tensor(dest[:, b, 0:ncol],
                                                     p[:, 0:ncol],
                                                     cbias_row[:, 0:ncol],
                                                     op=AOP.add)
                            first_drained.add(b)
                        else:
                            DV.wait_ge(dv.sem, dv.n)
                            d = DV.tensor_tensor(dest[:, b, 0:ncol],
                                                 dest[:, b, 0:ncol],
                                                 p[:, 0:ncol], op=AOP.add)
                        dv.inc(d)
                        psum_last[slot] = dv.n
                        if drain_seg_of_block[b] == si:
                            block_tail(b)
                if not chunks[cbase + CALL_CHUNKS - 1][3]:
                    pe.inc(mmi)
                pe_cons_vals.append(pe.n)
                gcall[0] += 1
            seg_end_hook(si)

    # ---- gating helpers ----
    g1w_b = None

    lam_copied = [0]   # dv value of the last lam psum drain

    def l1_tail(pass_no):
        gw = c_g1w if pass_no == 1 else c_g2w
        agg = agg1 if pass_no == 1 else agg2
        lm = lam["l1"] if pass_no == 1 else lam["l2"]

        def tail(b):
            # bias + relu fused on ACT (bias is per-partition: features)
            AC.wait_ge(dv.sem, dv.n)
            ac.inc(AC.activation(agg[:, b, :], agg[:, b, :], ACT.Relu,
                                 bias=c_b1c[:, 0:1]))
            # lam_b = relu(agg_b)^T g on the PE (contraction over features)
            PE.wait_ge(ac.sem, ac.n)
            if lam_copied[0]:
                PE.wait_ge(dv.sem, lam_copied[0])
            pe.inc(PE.matmul(lam_ps[:, b % 4:b % 4 + 1], agg[:, b, :], gw[:],
                             start=True, stop=True))
            if (b + 1) % 4 == 0 or b == NB - 1 or b == B1 - 1:
                c0 = b - b % 4
                DV.wait_ge(pe.sem, pe.n)
                dv.inc(DV.tensor_copy(lm[:, c0:b + 1],
                                      lam_ps[:, 0:b + 1 - c0]))
                lam_copied[0] = dv.n
        return tail

    ysent = {}

    def pass2_group(cs, ce):
        """sigmoid + normalize + combine + y table for block cols [cs, ce)."""
        AC.wait_ge(dv.sem, dv.n)
        AC.activation(lam["l1"][:, cs:ce], lam["l1"][:, cs:ce], ACT.Sigmoid,
                      bias=cbias[:, 0:1])
        ac.inc(AC.activation(lam["l2"][:, cs:ce], lam["l2"][:, cs:ce],
                             ACT.Sigmoid, bias=cbias[:, 1:2]))
        DV.wait_ge(ac.sem, ac.n)
        dv.inc(DV.tensor_tensor(lam["ls"][:, cs:ce], lam["l1"][:, cs:ce],
                                lam["l2"][:, cs:ce], op=AOP.add))
        DV.wait_ge(dv.sem, dv.n)
        dv.inc(DV.tensor_scalar(lam["ls"][:, cs:ce], lam["ls"][:, cs:ce],
                                1e-12, None, op0=AOP.max))
        DV.wait_ge(dv.sem, dv.n)
        dv.inc(DV.reciprocal(lam["ls"][:, cs:ce], lam["ls"][:, cs:ce]))
        DV.wait_ge(dv.sem, dv.n)
        dv.inc(DV.tensor_tensor(lam["w0"][:, cs:ce], lam["l1"][:, cs:ce],
                                lam["ls"][:, cs:ce], op=AOP.mult))
        dv.inc(DV.tensor_tensor(lam["w1"][:, cs:ce], lam["l2"][:, cs:ce],
                                lam["ls"][:, cs:ce], op=AOP.mult))
        comb = dv.n
        # y = x@W2 = w0*(x1@W2) + w1*(x2@W2): linearity lets the weights be
        # applied on the 32-wide y halves; aggT is already the matmul lhsT
        for b in range(cs, ce):
            i = b % 2
            if (b - 2) in ysent:
                PE.wait_ge(dv.sem, ysent[b - 2])
            PE.matmul(y_ps[i][:, 0:NCLASS], agg1[:, b, :], c_w2[:],
                      start=True, stop=True)
            pe.inc(PE.matmul(y_ps[i][:, NCLASS:2 * NCLASS], agg2[:, b, :],
                             c_w2[:], start=True, stop=True))
            w0c = lam["w0"][:, b:b + 1].broadcast_to([128, NCLASS])
            w1c = lam["w1"][:, b:b + 1].broadcast_to([128, NCLASS])
            DV.wait_ge(pe.sem, pe.n)
            DV.wait_ge(dv.sem, comb)
            dv.inc(DV.tensor_tensor(tmp[:, b, 0:NCLASS],
                                    y_ps[i][:, NCLASS:2 * NCLASS],
                                    w1c, op=AOP.mult))
            dv.inc(DV.tensor_tensor(ystage[:, b, 0:NCLASS],
                                    y_ps[i][:, 0:NCLASS],
                                    w0c, op=AOP.mult))
            DV.wait_ge(dv.sem, dv.n)
            dv.inc(DV.tensor_tensor(ystage[:, b, 0:NCLASS],
                                    ystage[:, b, 0:NCLASS],
                                    tmp[:, b, 0:NCLASS], op=AOP.add))
            ysent[b] = dv.n

    def fin_tail(pass_no):
        hw = c_h1w if pass_no == 1 else c_h2w
        pr = prop1 if pass_no == 1 else prop2
        lm = lam["l1"] if pass_no == 1 else lam["l2"]

        def tail(b):
            DV.wait_ge(dv.sem, dv.n)
            dv.inc(DV.tensor_tensor(outscr[:, b, 0:NCLASS], pr[:, b, :], hw[:],
                                    op=AOP.mult))
            DV.wait_ge(dv.sem, dv.n)
            dv.inc(DV.tensor_reduce(lm[:, b:b + 1], outscr[:, b, 0:NCLASS],
                                    axis=mybir.AxisListType.X, op=AOP.add))
        return tail

    def fin_group(cs, ce):
        AC.wait_ge(dv.sem, dv.n)
        AC.activation(lam["l1"][:, cs:ce], lam["l1"][:, cs:ce], ACT.Sigmoid,
                      bias=cbias[:, 2:3])
        ac.inc(AC.activation(lam["l2"][:, cs:ce], lam["l2"][:, cs:ce],
                             ACT.Sigmoid, bias=cbias[:, 3:4]))
        DV.wait_ge(ac.sem, ac.n)
        dv.inc(DV.tensor_tensor(lam["ls"][:, cs:ce], lam["l1"][:, cs:ce],
                                lam["l2"][:, cs:ce], op=AOP.add))
        DV.wait_ge(dv.sem, dv.n)
        dv.inc(DV.tensor_scalar(lam["ls"][:, cs:ce], lam["ls"][:, cs:ce],
                                1e-12, None, op0=AOP.max))
        DV.wait_ge(dv.sem, dv.n)
        dv.inc(DV.reciprocal(lam["ls"][:, cs:ce], lam["ls"][:, cs:ce]))
        DV.wait_ge(dv.sem, dv.n)
        dv.inc(DV.tensor_tensor(lam["w0"][:, cs:ce], lam["l1"][:, cs:ce],
                                lam["ls"][:, cs:ce], op=AOP.mult))
        dv.inc(DV.tensor_tensor(lam["w1"][:, cs:ce], lam["l2"][:, cs:ce],
                                lam["ls"][:, cs:ce], op=AOP.mult))
        w0b = lam["w0"][:, cs:ce, None].broadcast_to([128, ce - cs, NCLASS])
        w1b = lam["w1"][:, cs:ce, None].broadcast_to([128, ce - cs, NCLASS])
        DV.wait_ge(dv.sem, dv.n)
        dv.inc(DV.tensor_tensor(outscr[:, cs:ce, :], prop2[:, cs:ce, :], w1b,
                                op=AOP.mult))
        dv.inc(DV.tensor_tensor(outbuf[:, cs:ce, :], prop1[:, cs:ce, :], w0b,
                                op=AOP.mult))
        DV.wait_ge(dv.sem, dv.n)
        dv.inc(DV.tensor_tensor(outbuf[:, cs:ce, :], outbuf[:, cs:ce, :],
                                outscr[:, cs:ce, :], op=AOP.add))

    # =========== L1 pass 1 (adj1) ===========
    no_hook = lambda si: None
    drain_seg_p1 = {b: 1 for b in range(NB)}
    edge_pass(("l1", 1), (gidx_sb[0], dst_sb[0], ew_sb[0]), l1_tabs[1],
              256, 128, agg1, None, {0: cc01, 1: cc01}, {0: 0, 1: 0},
              l1_tail(1), no_hook, drain_seg_p1, transposed=True)
    pass1_gsem_vals = [(g.sem, g.n) for g in gsems]
    pass1_dv = dv.n

    # L2b idx loads on ACT into the L1a idx buffers (WAR: pass1 readers done)
    for s_, v_ in pass1_gsem_vals:
        AC.wait_ge(s_, v_)
    AC.wait_ge(dv.sem, pass1_dv)
    ns = meta[("l2", 2)][3]
    io2.inc(AC.dma_start(gidx_sb[0][:, 0:ns // 16], din["gidx_l22"][:]))
    io2.inc(AC.dma_start(dst_sb[0][:, 0:ns // 128], din["dst_l22"][:]))
    io2.inc(AC.dma_start(ew_sb[0][:, 0:ns // 128], din["eww_l22"][:]))
    idx_io[("l2", 2)] = io2.n
    fence_ac()

    # =========== L1 pass 2 (adj2), with pipelined gating ===========
    groups_half = {0: [(g, min(g + 4, B1)) for g in range(0, B1, 4)],
                   1: [(g, min(g + 4, NB)) for g in range(B1, NB, 4)]}
    tail2 = l1_tail(2)
    done_groups = set()

    def p2_tail(b):
        tail2(b)
        # fire group ops inline for the first half only; half-1 groups are
        # deferred past the last gather so their DVE work overlaps cc2A
        for gs, ge in groups_half[0]:
            if b == ge - 1 and (gs, ge) not in done_groups:
                done_groups.add((gs, ge))
                pass2_group(gs, ge)

    expand_io = {}
    t3_staged = {}

    def p2_seg_end(si):
        if si == 1:     # (0:B1, hi) done -> stage y[0:B1] to DRAM
            SY.wait_ge(dv.sem, ysent[B1 - 1])
            t3_staged[0] = io.inc(SY.dma_start(
                t3as_in[:].rearrange("(t p) f -> p t f", p=128),
                ystage[:, 0:B1, :]))
            fence_sp()


    drain_seg_p2 = {}
    for b in range(NB):
        drain_seg_p2[b] = 1 if b < B1 else 3
    edge_pass(("l1", 2), (gidx_sb[1], dst_sb[1], ew_sb[1]), l1_tabs[2],
              256, 128, agg2, None, {0: cc01, 1: cc01}, {0: 0, 1: 0},
              p2_tail, p2_seg_end, drain_seg_p2, transposed=True)
    for gs, ge in groups_half[1]:
        pass2_group(gs, ge)
    SY.wait_ge(dv.sem, ysent[NB - 1])
    t3_staged[1] = io.inc(SY.dma_start(
        t3bs_in[:].rearrange("(t p) f -> p t f", p=128),
        ystage[:, B1:NB, :]))
    fence_sp()

    # =========== cc2A + L2 window-A passes ===========
    GP.wait_ge(io.sem, t3_staged[0])
    cc.inc(gp_allgather(t3as_in[:], t3as_full[:]))
    SY.wait_ge(cc.sem, cc.n)
    expand_io[0] = io.inc(SY.dma_start(t3a_full[:, 0:NCLASS], t3as_full[:]))
    fence_sp()

    l2_cc = {0: 2, 1: 3}   # cc sem values after cc2A / cc2B
    drain_seg_l2 = {b: 1 for b in range(NB)}
    fd3, fd4 = set(), set()

    tailf = fin_tail(2)
    done_fgroups = set()
    fgroups = [(g, min(g + 4, NB)) for g in range(0, NB, 4)]

    def p4_tail(b):
        tailf(b)
        for gs, ge in fgroups:
            if b == ge - 1 and (gs, ge) not in done_fgroups:
                done_fgroups.add((gs, ge))
                fin_group(gs, ge)

    edge_pass(("l2", 1), (gidx_l2a, dst_l2a, ew_l2a), l2_tabs,
              None, NCLASS, prop1, c_b2r, l2_cc, expand_io,
              fin_tail(1), no_hook, drain_seg_l2, seg_range=(0,),
              first_drained=fd3)
    edge_pass(("l2", 2), (gidx_sb[0], dst_sb[0], ew_sb[0]), l2_tabs,
              None, NCLASS, prop2, c_b2r, l2_cc, expand_io,
              p4_tail, no_hook, drain_seg_l2, seg_range=(0,),
              first_drained=fd4)

    # =========== cc2B + L2 window-B passes ===========
    GP.wait_ge(io.sem, t3_staged[1])
    cc.inc(gp_allgather(t3bs_in[:], t3bs_full[:]))
    SY.wait_ge(cc.sem, cc.n)
    expand_io[1] = io.inc(SY.dma_start(t3b_full[:, 0:NCLASS], t3bs_full[:]))
    fence_sp()

    edge_pass(("l2", 1), (gidx_l2a, dst_l2a, ew_l2a), l2_tabs,
              None, NCLASS, prop1, c_b2r, l2_cc, expand_io,
              fin_tail(1), no_hook, drain_seg_l2, seg_range=(1,),
              first_drained=fd3)
    p3_dv = dv.n
    SY.wait_ge(dv.sem, p3_dv)
    io.inc(SY.dma_start(p1_o[:].rearrange("(t p) f -> p t f", p=128),
                        prop1[:, :, 0:NCLASS]))

    edge_pass(("l2", 2), (gidx_sb[0], dst_sb[0], ew_sb[0]), l2_tabs,
              None, NCLASS, prop2, c_b2r, l2_cc, expand_io,
              p4_tail, no_hook, drain_seg_l2, seg_range=(1,),
              first_drained=fd4)

    SY.wait_ge(dv.sem, dv.n)
    io.inc(SY.dma_start(out_o[:].rearrange("(t p) f -> p t f", p=128),
                        outbuf[:]))
    io.inc(SY.dma_start(p2_o[:].rearrange("(t p) f -> p t f", p=128),
                        prop2[:]))
    SY.wait_ge(io.sem, io.n)

    nc.compile()
    ctx.close()
    return nc


def _host_arrays(inputs, prep):
    bf = ml_dtypes.bfloat16
    f32 = np.float32

    def wfmt(w):  # [256, 64] -> [128, 2, 64] bf16
        return np.ascontiguousarray(
            np.asarray(w, f32).reshape(2, 128, NHID).transpose(1, 0, 2)).astype(bf)

    iota = np.tile(np.arange(128, dtype=f32), (128, 1))
    ident = np.eye(128, dtype=f32)
    g1w = np.asarray(inputs["g1w"], f32).reshape(128, 1)
    g2w = np.asarray(inputs["g2w"], f32).reshape(128, 1)
    h1w = np.tile(np.asarray(inputs["h1w"], f32).ravel(), (128, 1))
    h2w = np.tile(np.asarray(inputs["h2w"], f32).ravel(), (128, 1))
    b1c = np.concatenate([np.asarray(inputs["b1a"], f32).ravel(),
                          np.asarray(inputs["b1b"], f32).ravel()]).reshape(128, 1)
    b2r = np.tile(np.asarray(inputs["b2"], f32).ravel(), (128, 1))
    w2 = np.asarray(inputs["W2"], f32)  # [128, 32]

    common = dict(
        w1a=wfmt(inputs["W1a"]), w1b=wfmt(inputs["W1b"]),
        w2=w2.astype(bf), iota=iota.astype(bf), idb=ident.astype(bf),
        g1w=g1w.astype(bf), g2w=g2w.astype(bf),
        h1w=h1w.astype(bf), h2w=h2w.astype(bf),
        b1c=b1c, b2r=b2r)

    def xfmt(x, k):  # shard k, pad, transpose -> [128, 2, SP] bf16
        xs = np.asarray(x, f32)[k * S:(k + 1) * S]
        xp = np.zeros((SP, NFEAT), f32)
        xp[:S] = xs
        xt = xp.T.reshape(2, 128, SP).transpose(1, 0, 2)
        return np.ascontiguousarray(xt).astype(bf)

    in_maps = []
    for k in range(NCORES):
        m = dict(common)
        for v, key in (("xt1a", "x1a"), ("xt1b", "x1b"),
                       ("xt2a", "x2a"), ("xt2b", "x2b")):
            m[v] = xfmt(inputs[key], k)
        for lay in ("l1", "l2"):
            for a in (1, 2):
                g, d, e = prep[(lay, a)][0][k]
                m[f"gidx_{lay}{a}"] = g
                m[f"dst_{lay}{a}"] = d
                m[f"eww_{lay}{a}"] = e
        in_maps.append(m)
    return in_maps


def prepare(inputs):
    prep, segs = _prep_all(inputs)
    scalars = (float(np.asarray(inputs["g1b"]).ravel()[0]),
               float(np.asarray(inputs["g2b"]).ravel()[0]),
               float(np.asarray(inputs["h1b"]).ravel()[0]),
               float(np.asarray(inputs["h2b"]).ravel()[0]))
    nc = _build(prep, segs, scalars)
    in_maps = _host_arrays(inputs, prep)
    return nc, in_maps


LAST_EXEC_NS = None


def _run(inputs, sim=False):
    nc, in_maps = prepare(inputs)
    global LAST_EXEC_NS
    if sim:
        from concourse.bass_interp import MultiCoreSim
        msim = MultiCoreSim(nc, NCORES)
        for k in range(NCORES):
            for name, arr in in_maps[k].items():
                msim.cores[k].tensor(name)[:] = arr
            # pad cols 32:128 of the expanded L2 tables are fetched by the
            # 256B-row gathers but never consumed (matmul rhs is 32 wide);
            # zero them so the interp's NaN strictness check passes
            for t in ("t3a_full", "t3b_full"):
                msim.cores[k].tensor(t)[:] = 0
        msim.simulate()
        results = [{nm: msim.cores[k].tensor(nm).copy()
                    for nm in ("out_o", "p1_o", "p2_o")} for k in range(NCORES)]
    else:
        r = run_bass_kernel_spmd(nc, in_maps, list(range(NCORES)))
        LAST_EXEC_NS = r.exec_time_ns
        results = r.results

    outs = []
    for nm in ("out_o", "p1_o", "p2_o"):
        outs.append(np.concatenate([results[k][nm][:S] for k in range(NCORES)],
                    axis=0).astype(np.float32))
    return tuple(outs)


def kernel(**inputs):
    return _run(inputs, sim=False)
